# revision 23
# baseline (speedup 1.0000x reference)
"""Segment-mean over ragged contiguous segments of H, SPMD across 8 TRN2 NeuronCores.

out[g, :] = mean(H[start_g : start_g + sizes[g], :]), zero vector for empty segments.

Strategy (data-parallel over graphs, no cross-device communication):
  * Host: split graphs into 8 contiguous, row-balanced shards. Each core's rows are
    viewed as a stream of 128-row blocks; consecutive blocks are grouped into "tiles"
    such that every core's blocks in that tile span <= 128 segments.
  * Device (per block): build a one-hot matrix A[i, j] = (j == col_i) with a single
    VectorE tensor_scalar(is_equal) from per-row metadata, then TensorE matmul
    A.T @ H_block (f32r: fp32 storage at 1 cycle/row) accumulating into the tile's
    PSUM bank; the drain multiplies each segment row by 1/size. Segment raggedness
    lives entirely in the host-precomputed col metadata (tiny), so H is streamed
    exactly once at full DMA bandwidth.
  * Host: scatter per-(core, tile) slot ranges back to global segments, summing the
    partial results of segments that straddle a tile boundary.
"""
import numpy as np

P = 128          # partitions / rows per block
N_CORES = 8
D_EXPECTED = 300

_cache = {}


def _plan(sizes):
    """Compute the shared SPMD schedule + per-core metadata from segment sizes."""
    sizes = np.asarray(sizes, np.int64)
    G = sizes.shape[0]
    starts = np.zeros(G + 1, np.int64)
    np.cumsum(sizes, out=starts[1:])
    N = int(starts[-1])

    # contiguous graph ranges, balanced by rows
    bounds = [0]
    for c in range(1, N_CORES):
        target = (N * c) // N_CORES
        g = int(np.searchsorted(starts, target, side="left"))
        if g > 0 and (target - starts[g - 1]) < (starts[g] - target):
            g -= 1
        g = int(min(max(g, bounds[-1]), G))
        bounds.append(g)
    bounds.append(G)

    per_core = []
    rows_list = []
    for c in range(N_CORES):
        g0, g1 = bounds[c], bounds[c + 1]
        rows_c = int(starts[g1] - starts[g0])
        seg_ids = np.repeat(np.arange(g0, g1, dtype=np.int64), sizes[g0:g1])
        per_core.append({"g0": g0, "g1": g1, "rows": rows_c, "seg_ids": seg_ids,
                         "row0": int(starts[g0])})
        rows_list.append(rows_c)
    B = max((r + P - 1) // P for r in rows_list)

    # greedy tile partition: consecutive blocks while all cores' segment span <= 128
    def span(c, b0, b1):
        pc = per_core[c]
        lo = b0 * P
        hi = min(b1 * P, pc["rows"])
        if hi <= lo:
            return 0
        return int(pc["seg_ids"][hi - 1] - pc["seg_ids"][lo] + 1)

    MAX_KT = 36      # SBUF cap: 4 bufs x 36 blocks x 1200 B/partition
    kt = []          # blocks per tile
    tile_of_block = np.zeros(B, np.int64)
    b0 = 0
    while b0 < B:
        b1 = b0 + 1
        assert max(span(c, b0, b1) for c in range(N_CORES)) <= P, \
            "single block spans more than 128 segments"
        while (b1 < B and b1 - b0 < MAX_KT
               and max(span(c, b0, b1 + 1) for c in range(N_CORES)) <= P):
            b1 += 1
        tile_of_block[b0:b1] = len(kt)
        kt.append(b1 - b0)
        b0 = b1

    # Re-split the trailing blocks into small tiles so the end-of-kernel tail
    # (compute that can only start after the last big DMA lands) is short.
    # Splitting a tile only shrinks its segment span, so the <=128 constraint
    # still holds.
    tail_sizes = [8, 5, 3, 2, 1]     # resulting tile sizes at the end
    popped = 0
    while kt and popped < sum(tail_sizes):
        popped += kt.pop()
    new_tiles = []                   # built from the end backwards
    remaining = popped
    for s in tail_sizes[::-1]:       # smallest tile last
        if remaining <= 0:
            break
        s = min(s, remaining)
        new_tiles.append(s)
        remaining -= s
    while remaining > 0:
        new_tiles.append(min(MAX_KT, remaining))
        remaining -= new_tiles[-1]
    kt.extend(new_tiles[::-1])
    b = 0
    for ti, s in enumerate(kt):
        tile_of_block[b:b + s] = ti
        b += s
    assert b == B
    T = len(kt)
    tile_first_block = np.zeros(T, np.int64)
    np.cumsum(np.asarray(kt[:-1], np.int64), out=tile_first_block[1:])

    # per-core metadata: col (slot index per row), w (1/size per row), slot ranges
    inv_sizes = np.zeros(G, np.float32)
    nz = sizes > 0
    inv_sizes[nz] = (1.0 / sizes[nz].astype(np.float64)).astype(np.float32)
    for c in range(N_CORES):
        pc = per_core[c]
        rows_c = pc["rows"]
        seg_ids = pc["seg_ids"]
        # first segment touched per tile (for this core)
        first_seg = np.full(T, -1, np.int64)
        nslots = np.zeros(T, np.int64)
        for t in range(T):
            lo = int(tile_first_block[t]) * P
            hi = min((int(tile_first_block[t]) + kt[t]) * P, rows_c)
            if hi > lo:
                first_seg[t] = seg_ids[lo]
                nslots[t] = seg_ids[hi - 1] - seg_ids[lo] + 1
        col_flat = np.full(B * P, -1.0, np.float32)
        if rows_c > 0:
            row_tile = tile_of_block[np.arange(rows_c) // P]
            col_flat[:rows_c] = (seg_ids - first_seg[row_tile]).astype(np.float32)
        winv = np.zeros((P, T), np.float32)
        for t in range(T):
            ns = int(nslots[t])
            if ns > 0:
                fs = int(first_seg[t])
                winv[:ns, t] = inv_sizes[fs:fs + ns]
        pc["first_seg"] = first_seg
        pc["nslots"] = nslots
        pc["col"] = col_flat
        pc["winv"] = winv

    # per-tile output row count: max used slots over cores (trims out traffic)
    maxns = np.zeros(T, np.int64)
    for t in range(T):
        maxns[t] = max(int(per_core[c]["nslots"][t]) for c in range(N_CORES))
    out_off = np.zeros(T + 1, np.int64)
    np.cumsum(maxns, out=out_off[1:])

    return {"G": G, "N": N, "B": B, "T": T, "kt": kt,
            "tile_first_block": tile_first_block, "per_core": per_core,
            "maxns": maxns, "out_off": out_off}


def _build_program(plan, D):
    import concourse.bacc as bacc
    import concourse.mybir as mybir
    from concourse import tile

    B, T, kt = plan["B"], plan["T"], plan["kt"]
    tfb = plan["tile_first_block"]
    maxns = plan["maxns"]
    out_off = plan["out_off"]
    f32 = mybir.dt.float32
    f32r = mybir.dt.float32r

    nc = bacc.Bacc("TRN2", target_bir_lowering=False, debug=False,
                   num_devices=N_CORES)
    # f32r: fp32 storage, streams through the PE at 1 cycle/row (vs 4 for fp32)
    # planar layout: h[p, b*D:(b+1)*D] = H row (128*b + p) -> every partition
    # reads one fully contiguous run per tile DMA
    h = nc.declare_dram_parameter("h", [P, B * D], f32r, isOutput=False)
    # meta = [J (P cols) | col per block (B cols) | winv per tile (T cols)]
    meta = nc.declare_dram_parameter("meta", [P, P + B + T], f32, isOutput=False)
    out = nc.declare_dram_parameter("out", [int(out_off[T]), D], f32, isOutput=True)

    kmax = max(kt)
    with tile.TileContext(nc) as tc:
        with (
            tc.tile_pool(name="const", bufs=1) as cpool,
            tc.tile_pool(name="hbuf", bufs=3) as hpool,
            tc.tile_pool(name="abuf", bufs=2) as apool,
            tc.tile_pool(name="obuf", bufs=2) as opool,
            tc.tile_pool(name="psum", bufs=4, space="PSUM") as ppool,
        ):
            m_t = cpool.tile([P, P + B + T], f32)
            nc.scalar.dma_start(m_t[:], meta[:])
            j_t = m_t[:, 0:P]

            for t in range(T):
                k = kt[t]
                b0 = int(tfb[t])
                h_t = hpool.tile([P, kmax, D], f32r, tag="hbuf")
                nc.sync.dma_start(
                    h_t[:, 0:k, :],
                    h[:, b0 * D:(b0 + k) * D].rearrange("p (b d) -> p b d", d=D))
                # one-hot A for the whole tile in ONE DVE op:
                # A[i, b, j] = (J[i, j] == col[i, b]) via double-broadcast APs
                a_t = apool.tile([P, kmax, P], f32r, tag="abuf")
                j3 = j_t.unsqueeze(1).broadcast_to([P, k, P])
                c3 = m_t[:, P + b0:P + b0 + k].unsqueeze(2).broadcast_to([P, k, P])
                nc.vector.tensor_tensor(a_t[:, 0:k, :], j3, c3,
                                        mybir.AluOpType.is_equal)
                acc = ppool.tile([P, D], f32)
                for b in range(k):
                    nc.tensor.matmul(acc[:], a_t[:, b, :], h_t[:, b, :],
                                     start=(b == 0), stop=(b == k - 1))
                mn = int(maxns[t])
                o_t = opool.tile([P, D], f32, tag="obuf")
                winv = m_t[:, P + B + t:P + B + t + 1]
                nc.vector.tensor_scalar_mul(o_t[0:mn, :], acc[0:mn, :],
                                            winv[0:mn, :])
                # stores on the ACT HWDGE ring: keeps the Sync ring a pure
                # H-prefetch FIFO (a store waiting on its drain would
                # head-of-line block the next H load)
                nc.scalar.dma_start(
                    out[int(out_off[t]):int(out_off[t]) + mn, :], o_t[0:mn, :])
    nc.compile()
    return nc


def kernel(H, sizes):
    from concourse.bass_utils import run_bass_kernel_spmd

    H = np.ascontiguousarray(np.asarray(H, np.float32))
    sizes_np = np.asarray(sizes, np.int64)
    N, D = H.shape
    G = sizes_np.shape[0]

    key = (sizes_np.tobytes(), D)
    if key not in _cache:
        plan = _plan(sizes_np)
        assert plan["N"] == N, f"sizes sum {plan['N']} != H rows {N}"
        nc = _build_program(plan, D)
        _cache.clear()
        _cache[key] = (plan, nc)
    plan, nc = _cache[key]

    B, T = plan["B"], plan["T"]
    jmat = np.broadcast_to(np.arange(P, dtype=np.float32), (P, P))
    in_maps = []
    for c in range(N_CORES):
        pc = plan["per_core"][c]
        hpad = np.zeros((B * P, D), np.float32)
        hpad[:pc["rows"]] = H[pc["row0"]:pc["row0"] + pc["rows"]]
        # planar: [P, B*D] with h[p, b*D:(b+1)*D] = row 128*b + p
        hplanar = np.ascontiguousarray(
            hpad.reshape(B, P, D).transpose(1, 0, 2).reshape(P, B * D))
        meta = np.concatenate(
            [jmat, pc["col"].reshape(B, P).T, pc["winv"]], axis=1)
        in_maps.append({"h": hplanar, "meta": np.ascontiguousarray(meta)})

    import os, sys
    # tracing only works when the test harness has installed the NTFF hooks
    trace = bool(os.environ.get("KERNEL_TRACE")) and "antenv.axon_hooks" in sys.modules
    kw = {}
    if trace:
        kw = {"trace": True, "tmpdir": os.environ.get("KERNEL_TRACE_DIR") or None}
    res = run_bass_kernel_spmd(nc, in_maps, core_ids=list(range(N_CORES)), **kw)

    global LAST_EXEC_NS
    LAST_EXEC_NS = getattr(res, "exec_time_ns", None)

    out_off = plan["out_off"]
    out_full = np.zeros((G, D), np.float32)
    for c in range(N_CORES):
        pc = plan["per_core"][c]
        dev = res.results[c]["out"]
        for t in range(T):
            ns = int(pc["nslots"][t])
            if ns > 0:
                fs = int(pc["first_seg"][t])
                oo = int(out_off[t])
                out_full[fs:fs + ns] += dev[oo:oo + ns]
    return out_full


LAST_EXEC_NS = None


# revision 24
# speedup vs baseline: 1.0324x; 1.0324x over previous
"""Segment-mean over ragged contiguous segments of H, SPMD across 8 TRN2 NeuronCores.

out[g, :] = mean(H[start_g : start_g + sizes[g], :]), zero vector for empty segments.

Strategy (data-parallel over graphs, no cross-device communication):
  * Host: split graphs into 8 contiguous, row-balanced shards. Each core's rows are
    viewed as a stream of 128-row blocks; consecutive blocks are grouped into "tiles"
    such that every core's blocks in that tile span <= 128 segments.
  * Device (per block): build a one-hot matrix A[i, j] = (j == col_i) with a single
    VectorE tensor_scalar(is_equal) from per-row metadata, then TensorE matmul
    A.T @ H_block (f32r: fp32 storage at 1 cycle/row) accumulating into the tile's
    PSUM bank; the drain multiplies each segment row by 1/size. Segment raggedness
    lives entirely in the host-precomputed col metadata (tiny), so H is streamed
    exactly once at full DMA bandwidth.
  * Host: scatter per-(core, tile) slot ranges back to global segments, summing the
    partial results of segments that straddle a tile boundary.
"""
import numpy as np

P = 128          # partitions / rows per block
N_CORES = 8
D_EXPECTED = 300

_cache = {}


def _plan(sizes):
    """Compute the shared SPMD schedule + per-core metadata from segment sizes."""
    sizes = np.asarray(sizes, np.int64)
    G = sizes.shape[0]
    starts = np.zeros(G + 1, np.int64)
    np.cumsum(sizes, out=starts[1:])
    N = int(starts[-1])

    # contiguous graph ranges, balanced by rows
    bounds = [0]
    for c in range(1, N_CORES):
        target = (N * c) // N_CORES
        g = int(np.searchsorted(starts, target, side="left"))
        if g > 0 and (target - starts[g - 1]) < (starts[g] - target):
            g -= 1
        g = int(min(max(g, bounds[-1]), G))
        bounds.append(g)
    bounds.append(G)

    per_core = []
    rows_list = []
    for c in range(N_CORES):
        g0, g1 = bounds[c], bounds[c + 1]
        rows_c = int(starts[g1] - starts[g0])
        seg_ids = np.repeat(np.arange(g0, g1, dtype=np.int64), sizes[g0:g1])
        per_core.append({"g0": g0, "g1": g1, "rows": rows_c, "seg_ids": seg_ids,
                         "row0": int(starts[g0])})
        rows_list.append(rows_c)
    B = max((r + P - 1) // P for r in rows_list)

    # greedy tile partition: consecutive blocks while all cores' segment span <= 128
    def span(c, b0, b1):
        pc = per_core[c]
        lo = b0 * P
        hi = min(b1 * P, pc["rows"])
        if hi <= lo:
            return 0
        return int(pc["seg_ids"][hi - 1] - pc["seg_ids"][lo] + 1)

    MAX_KT = 36      # SBUF cap: 4 bufs x 36 blocks x 1200 B/partition
    kt = []          # blocks per tile
    tile_of_block = np.zeros(B, np.int64)
    b0 = 0
    while b0 < B:
        b1 = b0 + 1
        assert max(span(c, b0, b1) for c in range(N_CORES)) <= P, \
            "single block spans more than 128 segments"
        while (b1 < B and b1 - b0 < MAX_KT
               and max(span(c, b0, b1 + 1) for c in range(N_CORES)) <= P):
            b1 += 1
        tile_of_block[b0:b1] = len(kt)
        kt.append(b1 - b0)
        b0 = b1

    # Re-split the trailing blocks into small tiles so the end-of-kernel tail
    # (compute that can only start after the last big DMA lands) is short.
    # Splitting a tile only shrinks its segment span, so the <=128 constraint
    # still holds.
    tail_sizes = [8, 5, 3, 2, 1]     # resulting tile sizes at the end
    popped = 0
    while kt and popped < sum(tail_sizes):
        popped += kt.pop()
    new_tiles = []                   # built from the end backwards
    remaining = popped
    for s in tail_sizes[::-1]:       # smallest tile last
        if remaining <= 0:
            break
        s = min(s, remaining)
        new_tiles.append(s)
        remaining -= s
    while remaining > 0:
        new_tiles.append(min(MAX_KT, remaining))
        remaining -= new_tiles[-1]
    kt.extend(new_tiles[::-1])
    b = 0
    for ti, s in enumerate(kt):
        tile_of_block[b:b + s] = ti
        b += s
    assert b == B
    T = len(kt)
    tile_first_block = np.zeros(T, np.int64)
    np.cumsum(np.asarray(kt[:-1], np.int64), out=tile_first_block[1:])

    # per-core metadata: col (slot index per row), w (1/size per row), slot ranges
    inv_sizes = np.zeros(G, np.float32)
    nz = sizes > 0
    inv_sizes[nz] = (1.0 / sizes[nz].astype(np.float64)).astype(np.float32)
    for c in range(N_CORES):
        pc = per_core[c]
        rows_c = pc["rows"]
        seg_ids = pc["seg_ids"]
        # first segment touched per tile (for this core)
        first_seg = np.full(T, -1, np.int64)
        nslots = np.zeros(T, np.int64)
        for t in range(T):
            lo = int(tile_first_block[t]) * P
            hi = min((int(tile_first_block[t]) + kt[t]) * P, rows_c)
            if hi > lo:
                first_seg[t] = seg_ids[lo]
                nslots[t] = seg_ids[hi - 1] - seg_ids[lo] + 1
        col_flat = np.full(B * P, -1.0, np.float32)
        if rows_c > 0:
            row_tile = tile_of_block[np.arange(rows_c) // P]
            col_flat[:rows_c] = (seg_ids - first_seg[row_tile]).astype(np.float32)
        winv = np.zeros((P, T), np.float32)
        for t in range(T):
            ns = int(nslots[t])
            if ns > 0:
                fs = int(first_seg[t])
                winv[:ns, t] = inv_sizes[fs:fs + ns]
        pc["first_seg"] = first_seg
        pc["nslots"] = nslots
        pc["col"] = col_flat
        pc["winv"] = winv

    # per-tile output row count: max used slots over cores (trims out traffic)
    maxns = np.zeros(T, np.int64)
    for t in range(T):
        maxns[t] = max(int(per_core[c]["nslots"][t]) for c in range(N_CORES))
    out_off = np.zeros(T + 1, np.int64)
    np.cumsum(maxns, out=out_off[1:])

    return {"G": G, "N": N, "B": B, "T": T, "kt": kt,
            "tile_first_block": tile_first_block, "per_core": per_core,
            "maxns": maxns, "out_off": out_off}


def _build_program(plan, D):
    import concourse.bacc as bacc
    import concourse.mybir as mybir
    from concourse import tile

    B, T, kt = plan["B"], plan["T"], plan["kt"]
    tfb = plan["tile_first_block"]
    maxns = plan["maxns"]
    out_off = plan["out_off"]
    f32 = mybir.dt.float32
    f32r = mybir.dt.float32r

    nc = bacc.Bacc("TRN2", target_bir_lowering=False, debug=False,
                   num_devices=N_CORES)
    # f32r: fp32 storage, streams through the PE at 1 cycle/row (vs 4 for fp32)
    # planar layout: h[p, b*D:(b+1)*D] = H row (128*b + p) -> every partition
    # reads one fully contiguous run per tile DMA
    h = nc.declare_dram_parameter("h", [P, B * D], f32r, isOutput=False)
    # meta = [J (P cols) | col per block (B cols) | winv per tile (T cols)]
    meta = nc.declare_dram_parameter("meta", [P, P + B + T], f32, isOutput=False)
    out = nc.declare_dram_parameter("out", [int(out_off[T]), D], f32, isOutput=True)

    kmax = max(kt)
    with tile.TileContext(nc) as tc:
        with (
            tc.tile_pool(name="const", bufs=1) as cpool,
            tc.tile_pool(name="hbuf", bufs=3) as hpool,
            tc.tile_pool(name="abuf", bufs=2) as apool,
            tc.tile_pool(name="obuf", bufs=2) as opool,
            tc.tile_pool(name="psum", bufs=4, space="PSUM") as ppool,
        ):
            m_t = cpool.tile([P, P + B + T], f32)
            nc.scalar.dma_start(m_t[:], meta[:])
            j_t = m_t[:, 0:P]

            for t in range(T):
                k = kt[t]
                b0 = int(tfb[t])
                h_t = hpool.tile([P, kmax, D], f32r, tag="hbuf")
                nc.sync.dma_start(
                    h_t[:, 0:k, :],
                    h[:, b0 * D:(b0 + k) * D].rearrange("p (b d) -> p b d", d=D))
                # one-hot A for the whole tile in ONE DVE op:
                # A[i, b, j] = (J[i, j] == col[i, b]) via double-broadcast APs
                a_t = apool.tile([P, kmax, P], f32r, tag="abuf")
                j3 = j_t.unsqueeze(1).broadcast_to([P, k, P])
                c3 = m_t[:, P + b0:P + b0 + k].unsqueeze(2).broadcast_to([P, k, P])
                nc.vector.tensor_tensor(a_t[:, 0:k, :], j3, c3,
                                        mybir.AluOpType.is_equal)
                acc = ppool.tile([P, D], f32)
                for b in range(k):
                    nc.tensor.matmul(acc[:], a_t[:, b, :], h_t[:, b, :],
                                     start=(b == 0), stop=(b == k - 1))
                mn = int(maxns[t])
                o_t = opool.tile([P, D], f32, tag="obuf")
                winv = m_t[:, P + B + t:P + B + t + 1]
                # drain on ScalarE (out = in*scale): keeps the DVE stream a pure
                # A-build pipeline (a DVE drain would queue the next tile's
                # A-build behind this tile's matmuls)
                nc.scalar.activation(o_t[0:mn, :], acc[0:mn, :],
                                     mybir.ActivationFunctionType.Copy,
                                     scale=winv[0:mn, :])
                # stores on the ACT HWDGE ring: keeps the Sync ring a pure
                # H-prefetch FIFO (a store waiting on its drain would
                # head-of-line block the next H load)
                nc.scalar.dma_start(
                    out[int(out_off[t]):int(out_off[t]) + mn, :], o_t[0:mn, :])
    nc.compile()
    return nc


def kernel(H, sizes):
    from concourse.bass_utils import run_bass_kernel_spmd

    H = np.ascontiguousarray(np.asarray(H, np.float32))
    sizes_np = np.asarray(sizes, np.int64)
    N, D = H.shape
    G = sizes_np.shape[0]

    key = (sizes_np.tobytes(), D)
    if key not in _cache:
        plan = _plan(sizes_np)
        assert plan["N"] == N, f"sizes sum {plan['N']} != H rows {N}"
        nc = _build_program(plan, D)
        _cache.clear()
        _cache[key] = (plan, nc)
    plan, nc = _cache[key]

    B, T = plan["B"], plan["T"]
    jmat = np.broadcast_to(np.arange(P, dtype=np.float32), (P, P))
    in_maps = []
    for c in range(N_CORES):
        pc = plan["per_core"][c]
        hpad = np.zeros((B * P, D), np.float32)
        hpad[:pc["rows"]] = H[pc["row0"]:pc["row0"] + pc["rows"]]
        # planar: [P, B*D] with h[p, b*D:(b+1)*D] = row 128*b + p
        hplanar = np.ascontiguousarray(
            hpad.reshape(B, P, D).transpose(1, 0, 2).reshape(P, B * D))
        meta = np.concatenate(
            [jmat, pc["col"].reshape(B, P).T, pc["winv"]], axis=1)
        in_maps.append({"h": hplanar, "meta": np.ascontiguousarray(meta)})

    import os, sys
    # tracing only works when the test harness has installed the NTFF hooks
    trace = bool(os.environ.get("KERNEL_TRACE")) and "antenv.axon_hooks" in sys.modules
    kw = {}
    if trace:
        kw = {"trace": True, "tmpdir": os.environ.get("KERNEL_TRACE_DIR") or None}
    res = run_bass_kernel_spmd(nc, in_maps, core_ids=list(range(N_CORES)), **kw)

    global LAST_EXEC_NS
    LAST_EXEC_NS = getattr(res, "exec_time_ns", None)

    out_off = plan["out_off"]
    out_full = np.zeros((G, D), np.float32)
    for c in range(N_CORES):
        pc = plan["per_core"][c]
        dev = res.results[c]["out"]
        for t in range(T):
            ns = int(pc["nslots"][t])
            if ns > 0:
                fs = int(pc["first_seg"][t])
                oo = int(out_off[t])
                out_full[fs:fs + ns] += dev[oo:oo + ns]
    return out_full


LAST_EXEC_NS = None


# revision 27
# speedup vs baseline: 1.1457x; 1.1097x over previous
"""Segment-mean over ragged contiguous segments of H, SPMD across 8 TRN2 NeuronCores.

out[g, :] = mean(H[start_g : start_g + sizes[g], :]), zero vector for empty segments.

Strategy (data-parallel over graphs, no cross-device communication):
  * Host: split graphs into 8 contiguous, row-balanced shards. Each core's rows are
    viewed as a stream of 128-row blocks; consecutive blocks are grouped into "tiles"
    such that every core's blocks in that tile span <= 128 segments.
  * Device (per block): build a one-hot matrix A[i, j] = (j == col_i) with a single
    VectorE tensor_scalar(is_equal) from per-row metadata, then TensorE matmul
    A.T @ H_block (f32r: fp32 storage at 1 cycle/row) accumulating into the tile's
    PSUM bank; the drain multiplies each segment row by 1/size. Segment raggedness
    lives entirely in the host-precomputed col metadata (tiny), so H is streamed
    exactly once at full DMA bandwidth.
  * Host: scatter per-(core, tile) slot ranges back to global segments, summing the
    partial results of segments that straddle a tile boundary.
"""
import numpy as np

P = 128          # partitions / rows per block
N_CORES = 8
D_EXPECTED = 300

_cache = {}


def _plan(sizes):
    """Compute the shared SPMD schedule + per-core metadata from segment sizes."""
    sizes = np.asarray(sizes, np.int64)
    G = sizes.shape[0]
    starts = np.zeros(G + 1, np.int64)
    np.cumsum(sizes, out=starts[1:])
    N = int(starts[-1])

    # contiguous graph ranges, balanced by rows
    bounds = [0]
    for c in range(1, N_CORES):
        target = (N * c) // N_CORES
        g = int(np.searchsorted(starts, target, side="left"))
        if g > 0 and (target - starts[g - 1]) < (starts[g] - target):
            g -= 1
        g = int(min(max(g, bounds[-1]), G))
        bounds.append(g)
    bounds.append(G)

    per_core = []
    rows_list = []
    for c in range(N_CORES):
        g0, g1 = bounds[c], bounds[c + 1]
        rows_c = int(starts[g1] - starts[g0])
        seg_ids = np.repeat(np.arange(g0, g1, dtype=np.int64), sizes[g0:g1])
        per_core.append({"g0": g0, "g1": g1, "rows": rows_c, "seg_ids": seg_ids,
                         "row0": int(starts[g0])})
        rows_list.append(rows_c)
    B = max((r + P - 1) // P for r in rows_list)

    # greedy tile partition: consecutive blocks while all cores' segment span <= 128
    def span(c, b0, b1):
        pc = per_core[c]
        lo = b0 * P
        hi = min(b1 * P, pc["rows"])
        if hi <= lo:
            return 0
        return int(pc["seg_ids"][hi - 1] - pc["seg_ids"][lo] + 1)

    MAX_KT = 36      # SBUF cap: 4 bufs x 36 blocks x 1200 B/partition
    kt = []          # blocks per tile
    tile_of_block = np.zeros(B, np.int64)
    b0 = 0
    while b0 < B:
        b1 = b0 + 1
        assert max(span(c, b0, b1) for c in range(N_CORES)) <= P, \
            "single block spans more than 128 segments"
        while (b1 < B and b1 - b0 < MAX_KT
               and max(span(c, b0, b1 + 1) for c in range(N_CORES)) <= P):
            b1 += 1
        tile_of_block[b0:b1] = len(kt)
        kt.append(b1 - b0)
        b0 = b1

    # Re-split the trailing blocks into small tiles so the end-of-kernel tail
    # (compute that can only start after the last big DMA lands) is short.
    # Splitting a tile only shrinks its segment span, so the <=128 constraint
    # still holds.
    tail_sizes = [8, 5, 3, 2, 1]     # resulting tile sizes at the end
    popped = 0
    while kt and popped < sum(tail_sizes):
        popped += kt.pop()
    new_tiles = []                   # built from the end backwards
    remaining = popped
    for s in tail_sizes[::-1]:       # smallest tile last
        if remaining <= 0:
            break
        s = min(s, remaining)
        new_tiles.append(s)
        remaining -= s
    while remaining > 0:
        new_tiles.append(min(MAX_KT, remaining))
        remaining -= new_tiles[-1]
    kt.extend(new_tiles[::-1])
    b = 0
    for ti, s in enumerate(kt):
        tile_of_block[b:b + s] = ti
        b += s
    assert b == B
    T = len(kt)
    tile_first_block = np.zeros(T, np.int64)
    np.cumsum(np.asarray(kt[:-1], np.int64), out=tile_first_block[1:])

    # per-core metadata: col (slot index per row), w (1/size per row), slot ranges
    inv_sizes = np.zeros(G, np.float32)
    nz = sizes > 0
    inv_sizes[nz] = (1.0 / sizes[nz].astype(np.float64)).astype(np.float32)
    for c in range(N_CORES):
        pc = per_core[c]
        rows_c = pc["rows"]
        seg_ids = pc["seg_ids"]
        # first segment touched per tile (for this core)
        first_seg = np.full(T, -1, np.int64)
        nslots = np.zeros(T, np.int64)
        for t in range(T):
            lo = int(tile_first_block[t]) * P
            hi = min((int(tile_first_block[t]) + kt[t]) * P, rows_c)
            if hi > lo:
                first_seg[t] = seg_ids[lo]
                nslots[t] = seg_ids[hi - 1] - seg_ids[lo] + 1
        col_flat = np.full(B * P, -1.0, np.float32)
        if rows_c > 0:
            row_tile = tile_of_block[np.arange(rows_c) // P]
            col_flat[:rows_c] = (seg_ids - first_seg[row_tile]).astype(np.float32)
        winv = np.zeros((P, T), np.float32)
        for t in range(T):
            ns = int(nslots[t])
            if ns > 0:
                fs = int(first_seg[t])
                winv[:ns, t] = inv_sizes[fs:fs + ns]
        pc["first_seg"] = first_seg
        pc["nslots"] = nslots
        pc["col"] = col_flat
        pc["winv"] = winv

    # per-tile output row count: max used slots over cores (trims out traffic)
    maxns = np.zeros(T, np.int64)
    for t in range(T):
        maxns[t] = max(int(per_core[c]["nslots"][t]) for c in range(N_CORES))
    out_off = np.zeros(T + 1, np.int64)
    np.cumsum(maxns, out=out_off[1:])

    return {"G": G, "N": N, "B": B, "T": T, "kt": kt,
            "tile_first_block": tile_first_block, "per_core": per_core,
            "maxns": maxns, "out_off": out_off}


def _build_program(plan, D):
    import concourse.bacc as bacc
    import concourse.mybir as mybir
    from concourse import tile

    B, T, kt = plan["B"], plan["T"], plan["kt"]
    tfb = plan["tile_first_block"]
    f32 = mybir.dt.float32
    f32r = mybir.dt.float32r

    nc = bacc.Bacc("TRN2", target_bir_lowering=False, debug=False,
                   num_devices=N_CORES)
    # f32r: fp32 storage, streams through the PE at 1 cycle/row (vs 4 for fp32)
    # planar layout: h[p, b*D:(b+1)*D] = H row (128*b + p) -> every partition
    # reads one fully contiguous run per tile DMA
    h = nc.declare_dram_parameter("h", [P, B * D], f32r, isOutput=False)
    # meta = [J (P cols) | col per block (B cols) | winv per tile (T cols)]
    meta = nc.declare_dram_parameter("meta", [P, P + B + T], f32, isOutput=False)
    out = nc.declare_dram_parameter("out", [T * P, D], f32, isOutput=True)

    kmax = max(kt)
    with tile.TileContext(nc) as tc:
        with (
            tc.tile_pool(name="const", bufs=1) as cpool,
            tc.tile_pool(name="hbuf", bufs=4) as hpool,
            tc.tile_pool(name="abuf", bufs=8) as apool,
            tc.tile_pool(name="obuf", bufs=2) as opool,
            tc.tile_pool(name="psum", bufs=4, space="PSUM") as ppool,
        ):
            m_t = cpool.tile([P, P + B + T], f32)
            nc.scalar.dma_start(m_t[:], meta[:])
            j_t = m_t[:, 0:P]

            for t in range(T):
                k = kt[t]
                b0 = int(tfb[t])
                h_t = hpool.tile([P, kmax, D], f32r, tag="hbuf")
                nc.sync.dma_start(
                    h_t[:, 0:k, :],
                    h[:, b0 * D:(b0 + k) * D].rearrange("p (b d) -> p b d", d=D))
                acc = ppool.tile([P, D], f32)
                for b in range(k):
                    g = b0 + b
                    a_t = apool.tile([P, P], f32r, tag="abuf")
                    col = m_t[:, P + g:P + g + 1]
                    nc.vector.tensor_scalar(
                        a_t[:], j_t, col, None, mybir.AluOpType.is_equal)
                    nc.tensor.matmul(acc[:], a_t[:], h_t[:, b, :],
                                     start=(b == 0), stop=(b == k - 1))
                o_t = opool.tile([P, D], f32, tag="obuf")
                winv = m_t[:, P + B + t:P + B + t + 1]
                nc.vector.tensor_scalar_mul(o_t[:], acc[:], winv)
                # stores on the ACT HWDGE ring: keeps the Sync ring a pure
                # H-prefetch FIFO (a store waiting on its drain would
                # head-of-line block the next H load)
                nc.scalar.dma_start(out[t * P:(t + 1) * P, :], o_t[:])
    nc.compile()
    return nc


def kernel(H, sizes):
    from concourse.bass_utils import run_bass_kernel_spmd

    H = np.ascontiguousarray(np.asarray(H, np.float32))
    sizes_np = np.asarray(sizes, np.int64)
    N, D = H.shape
    G = sizes_np.shape[0]

    key = (sizes_np.tobytes(), D)
    if key not in _cache:
        plan = _plan(sizes_np)
        assert plan["N"] == N, f"sizes sum {plan['N']} != H rows {N}"
        nc = _build_program(plan, D)
        _cache.clear()
        _cache[key] = (plan, nc)
    plan, nc = _cache[key]

    B, T = plan["B"], plan["T"]
    jmat = np.broadcast_to(np.arange(P, dtype=np.float32), (P, P))
    in_maps = []
    for c in range(N_CORES):
        pc = plan["per_core"][c]
        hpad = np.zeros((B * P, D), np.float32)
        hpad[:pc["rows"]] = H[pc["row0"]:pc["row0"] + pc["rows"]]
        # planar: [P, B*D] with h[p, b*D:(b+1)*D] = row 128*b + p
        hplanar = np.ascontiguousarray(
            hpad.reshape(B, P, D).transpose(1, 0, 2).reshape(P, B * D))
        meta = np.concatenate(
            [jmat, pc["col"].reshape(B, P).T, pc["winv"]], axis=1)
        in_maps.append({"h": hplanar, "meta": np.ascontiguousarray(meta)})

    import os, sys
    # tracing only works when the test harness has installed the NTFF hooks
    trace = bool(os.environ.get("KERNEL_TRACE")) and "antenv.axon_hooks" in sys.modules
    kw = {}
    if trace:
        kw = {"trace": True, "tmpdir": os.environ.get("KERNEL_TRACE_DIR") or None}
    res = run_bass_kernel_spmd(nc, in_maps, core_ids=list(range(N_CORES)), **kw)

    global LAST_EXEC_NS
    LAST_EXEC_NS = getattr(res, "exec_time_ns", None)

    out_full = np.zeros((G, D), np.float32)
    for c in range(N_CORES):
        pc = plan["per_core"][c]
        dev = res.results[c]["out"]
        for t in range(T):
            ns = int(pc["nslots"][t])
            if ns > 0:
                fs = int(pc["first_seg"][t])
                out_full[fs:fs + ns] += dev[t * P:t * P + ns]
    return out_full


LAST_EXEC_NS = None
